# revision 8
# baseline (speedup 1.0000x reference)
"""CodeGen attention on 8 Trainium2 NeuronCores (Bass/Tile).

Sharding: tensor-parallel over the 4 CodeGen mp head-groups x data-parallel
over batch 2. Core c = dp*4 + tp handles batch dp, head group tp (4 heads).

Per-core pipeline (all matmuls fp32r):
  phase 1: QKV projection. Q^T/K^T produced transposed (W stationary,
           X^T moving) with rotary fused on the first 64 rows of each head;
           V produced in natural [s, d] layout (X^T stationary, Wv moving).
  phase 2: causal attention per head. scores^T tiles, softmax without
           max-subtraction (logits are O(5)), column sums via an ones-vector
           matmul, PV accumulated in PSUM, late normalize by 1/rowsum
           broadcast through a K=1 matmul. The inner ki loop is software-
           pipelined: scores(ki+1) is emitted before rowsum/PV(ki) so the
           Tensor queue never stalls on the exp/mask chain.
  Per-head chunked AllGather (within each batch group of 4), each chunk
  issued as soon as its head finishes so 3/4 of the collective overlaps
  attention compute. og_out is head-major; the host permutes W_out rows
  to compensate.
  phase 3: out-projection against this core's 1024-column slice of W_out.

Host assembles the [B, S, D] output from per-core [S, D/4] column shards.
"""

import numpy as np

B, S, D = 2, 2048, 4096
N_HEAD = 16
HD = 256
MP = 4
ROT = 64
LOCAL = D // MP            # 1024 (= 4 heads * 256)
QK_TILES = 2 * LOCAL // 128  # 16: q col-tiles then k col-tiles
DT = D // 128              # 32 contraction tiles
N_CORES = 8
H_LOC = N_HEAD // MP       # 4 heads per core

_CACHE = {}


def _emit_body(nc, tc, tens, psp, cpool, with_collective, rep, phases=(1, 2, 3)):
    """One full pipeline pass (phases 1-3). rep only namespaces DRAM scratch."""
    import concourse.tile as tile  # noqa: F401
    from concourse import mybir

    f32 = mybir.dt.float32
    f32r = mybir.dt.float32r
    bf16 = mybir.dt.bfloat16
    EXP = mybir.ActivationFunctionType.Exp

    (xt_r, wq_r, wk_r, wv_r, wo_r, cost, sint, masks, y,
     qT_d, kT_d, v_d, og_in, og_out, rt_sb, ones_sb, onesr_sb) = tens

    # ---------------- phase 1: QKV ----------------
    if 1 in phases:
     with tc.tile_pool(name="xrp", bufs=1) as xrp, \
         tc.tile_pool(name="wmp", bufs=2) as wmp, \
         tc.tile_pool(name="sqp", bufs=3) as sqp, \
         tc.tile_pool(name="t1p", bufs=2) as t1p, \
         tc.tile_pool(name="t2p", bufs=2) as t2p, \
         tc.tile_pool(name="wvp", bufs=3) as wvp, \
         tc.tile_pool(name="trig", bufs=1) as trig:
        cost_sb = trig.tile([ROT, S], bf16, name="cost_sb")
        nc.sync.dma_start(cost_sb[:], cost.ap())
        sint_sb = trig.tile([ROT, S], bf16, name="sint_sb")
        nc.sync.dma_start(sint_sb[:], sint.ap())

        for ss in range(2):  # 1024-wide s slices
            xr = xrp.tile([128, DT, 1024], f32r, name="xr", tag="xr")
            for dt in range(DT):
                nc.sync.dma_start(
                    xr[:, dt, :], xt_r[:, dt, ss * 1024:(ss + 1) * 1024])

            # Q^T / K^T (W stationary, X^T moving), rotary fused
            for m in range(QK_TILES):
                w_r = wq_r if m < 8 else wk_r
                col0 = (m % 8) * 128
                wm = wmp.tile([128, DT, 128], f32r, name="wm", tag="wm")
                nc.sync.dma_start(wm[:], w_r[:, :, col0:col0 + 128])
                for n in range(2):
                    ps = psp.tile([128, 512], f32, name="ps", tag="ps")
                    for dt in range(DT):
                        nc.tensor.matmul(
                            ps[:], wm[:, dt, :],
                            xr[:, dt, n * 512:(n + 1) * 512],
                            start=(dt == 0), stop=(dt == DT - 1))
                    sg = ss * 1024 + n * 512
                    dest = qT_d if m < 8 else kT_d
                    sq = sqp.tile([128, 512], f32r, name="sq", tag="sq")
                    nc.vector.tensor_copy(sq[:], ps[:])
                    if m % 2 == 0:
                        # rows 0:64 are the rotary dims of a head
                        rp = psp.tile([128, 512], f32, name="rp", tag="ps")
                        nc.tensor.matmul(rp[0:ROT, :], rt_sb[:],
                                         sq[0:ROT, :], start=True, stop=True)
                        t1 = t1p.tile([ROT, 512], f32, name="t1", tag="t1")
                        nc.vector.tensor_mul(t1[:], ps[0:ROT, :],
                                             cost_sb[:, sg:sg + 512])
                        t2 = t2p.tile([ROT, 512], f32, name="t2", tag="t2")
                        nc.vector.tensor_mul(t2[:], rp[0:ROT, :],
                                             sint_sb[:, sg:sg + 512])
                        nc.vector.tensor_add(sq[0:ROT, :], t1[:], t2[:])
                    nc.sync.dma_start(
                        dest[col0:col0 + 128, sg:sg + 512], sq[:])

            # V natural layout (X^T stationary, Wv moving)
            for vn in range(2):
                pss = [psp.tile([128, 512], f32, name=f"vps{sm}", tag="ps")
                       for sm in range(8)]
                for dt in range(DT):
                    wvt = wvp.tile([128, 512], f32r, name="wvt", tag="wvt")
                    nc.sync.dma_start(
                        wvt[:], wv_r[:, dt, vn * 512:(vn + 1) * 512])
                    for sm in range(8):
                        nc.tensor.matmul(
                            pss[sm][:], xr[:, dt, sm * 128:(sm + 1) * 128],
                            wvt[:], start=(dt == 0), stop=(dt == DT - 1))
                for sm in range(8):
                    vc = sqp.tile([128, 512], f32r, name="vc", tag="sq")
                    nc.vector.tensor_copy(vc[:], pss[sm][:])
                    nc.sync.dma_start(
                        v_d[ss * 1024 + sm * 128:ss * 1024 + (sm + 1) * 128,
                            vn * 512:(vn + 1) * 512], vc[:])

    # ---------------- phase 2: attention ----------------
    if 2 in phases:
     with tc.tile_pool(name="qtp", bufs=2) as qtp, \
         tc.tile_pool(name="ktp", bufs=2) as ktp, \
         tc.tile_pool(name="vtp", bufs=2) as vtp, \
         tc.tile_pool(name="etp", bufs=6) as etp, \
         tc.tile_pool(name="etmp", bufs=3) as etmp, \
         tc.tile_pool(name="otp", bufs=2) as otp, \
         tc.tile_pool(name="rbp", bufs=2) as rbp, \
         tc.tile_pool(name="rip", bufs=2) as rip, \
         tc.tile_pool(name="mkp", bufs=1) as mkp:
        masks_sb = mkp.tile([128, 4, 512], f32, name="masks_sb")
        nc.sync.dma_start(masks_sb[:], masks.ap())

        for h in range(H_LOC):
            r0 = h * HD
            qt = qtp.tile([128, 2, S], f32r, name="qt", tag="qt")
            qsrc = qT_d[r0:r0 + HD, :].rearrange("(dd p) s -> p dd s", p=128)
            for dd in range(2):
                nc.sync.dma_start(qt[:, dd, :], qsrc[:, dd, :])
            kt = ktp.tile([128, 2, S], f32r, name="kt", tag="kt")
            ksrc = kT_d[r0:r0 + HD, :].rearrange("(dd p) s -> p dd s", p=128)
            for dd in range(2):
                nc.sync.dma_start(kt[:, dd, :], ksrc[:, dd, :])
            vt = vtp.tile([128, S // 128, HD], f32r, name="vt", tag="vt")
            vsrc = v_d[:, r0:r0 + HD].rearrange("(st p) c -> p st c", p=128)
            for st in range(S // 128):
                nc.sync.dma_start(vt[:, st, :], vsrc[:, st, :])
            ot = otp.tile([128, 2, S], f32r, name="ot", tag="ot")

            for qn in range(4):
                nk = (qn + 1) * 4
                q0 = qn * 512
                rs = psp.tile([1, 512], f32, name="rs", tag="ps")
                ov = [psp.tile([128, 512], f32, name=f"ov{dm}", tag="ps")
                      for dm in range(2)]

                n_cons = [0]

                def _consume(et, ki):
                    # start/stop keyed to Tensor-queue emission order, not ki
                    st, sp_ = n_cons[0] == 0, n_cons[0] == nk - 1
                    n_cons[0] += 1
                    nc.tensor.matmul(rs[:], ones_sb[:], et[:],
                                     start=st, stop=sp_)
                    for dm in range(2):
                        nc.tensor.matmul(
                            ov[dm][:], vt[:, ki, dm * 128:(dm + 1) * 128],
                            et[:], start=st, stop=sp_)

                # depth-2 software pipeline, masked (diagonal) blocks first:
                # their exp->mask 2-op chain gets two score-batches of slack
                # before the consume lands on the Tensor queue.
                order = list(range(qn * 4, nk)) + list(range(0, qn * 4))
                pend = []
                for ki in order:
                    sp = psp.tile([128, 512], f32, name="sp", tag="ps")
                    for dd in range(2):
                        nc.tensor.matmul(
                            sp[:], kt[:, dd, ki * 128:(ki + 1) * 128],
                            qt[:, dd, q0:q0 + 512],
                            start=(dd == 0), stop=(dd == 1))
                    if len(pend) >= 2:
                        _consume(*pend.pop(0))
                    et = etp.tile([128, 512], f32r, name="et", tag="et")
                    if ki >= qn * 4:
                        etm = etmp.tile([128, 512], f32, name="etm", tag="etm")
                        nc.scalar.activation(etm[:], sp[:], EXP,
                                             bias=0.0, scale=1.0 / 16.0)
                        nc.vector.tensor_mul(et[:], etm[:],
                                             masks_sb[:, ki - qn * 4, :])
                    else:
                        nc.scalar.activation(et[:], sp[:], EXP,
                                             bias=0.0, scale=1.0 / 16.0)
                    pend.append((et, ki))
                for item in pend:
                    _consume(*item)
                rinv = rip.tile([1, 512], f32r, name="rinv", tag="rinv")
                # f32r is bit-identical to f32 here; only the matmul
                # datapath reads it differently.
                with nc.allow_low_precision(reason="f32r == f32 bits"):
                    nc.vector.reciprocal(rinv[:], rs[:])
                rb = psp.tile([128, 512], f32, name="rb", tag="ps")
                nc.tensor.matmul(rb[:], onesr_sb[:], rinv[:],
                                 start=True, stop=True)
                rb_sb = rbp.tile([128, 512], f32, name="rb_sb", tag="rb")
                nc.vector.tensor_copy(rb_sb[:], rb[:])
                for dm in range(2):
                    nc.vector.tensor_mul(ot[:, dm, q0:q0 + 512],
                                         ov[dm][:], rb_sb[:])
            for dm in range(2):
                rr = (h * 2 + dm) * 128
                nc.sync.dma_start(og_in[rr:rr + 128, :], ot[:, dm, :])

                # chunked AllGather per (head, dm), overlapped with the
                # remaining attention. og_out rows are chunk-major
                # (c*512 + tp*128 + i, c = h*2+dm); the host permutes
                # W_out rows to match.
                if 3 in phases:
                    c = h * 2 + dm
                    if with_collective:
                        nc.gpsimd.collective_compute(
                            "AllGather",
                            mybir.AluOpType.bypass,
                            replica_groups=[[0, 1, 2, 3], [4, 5, 6, 7]],
                            ins=[og_in[c * 128:(c + 1) * 128, :].opt()],
                            outs=[og_out[c * 512:(c + 1) * 512, :].opt()],
                        )
                    else:
                        for blk in range(MP):
                            nc.sync.dma_start(
                                og_out[c * 512 + blk * 128:
                                       c * 512 + (blk + 1) * 128, :],
                                og_in[c * 128:(c + 1) * 128, :])

    if 3 not in phases:
        return

    # ---------------- phase 3: out projection ----------------
    og_r = og_out[:].rearrange("(dt p) s -> p dt s", p=128)
    with tc.tile_pool(name="wop", bufs=1) as wop, \
         tc.tile_pool(name="omp", bufs=3) as omp, \
         tc.tile_pool(name="resp", bufs=4) as resp:
        wo_sb = wop.tile([128, DT, LOCAL], f32r, name="wo_sb")
        for dt in range(DT):
            nc.sync.dma_start(wo_sb[:, dt, :], wo_r[:, dt, :])
        for qm in range(16):
            om = omp.tile([128, DT, 128], f32r, name="om", tag="om")
            for db in range(4):
                nc.sync.dma_start(
                    om[:, db * 8:(db + 1) * 8, :],
                    og_r[:, db * 8:(db + 1) * 8, qm * 128:(qm + 1) * 128])
            for cn in range(2):
                ps3 = psp.tile([128, 512], f32, name="ps3", tag="ps")
                for dt in range(DT):
                    nc.tensor.matmul(
                        ps3[:], om[:, dt, :],
                        wo_sb[:, dt, cn * 512:(cn + 1) * 512],
                        start=(dt == 0), stop=(dt == DT - 1))
                res = resp.tile([128, 512], f32, name="res", tag="res")
                nc.vector.tensor_copy(res[:], ps3[:])
                nc.sync.dma_start(
                    y.ap()[qm * 128:(qm + 1) * 128,
                           cn * 512:(cn + 1) * 512], res[:])


def _build_program(with_collective=True, n_repeat=1):
    import concourse.bass as bass  # noqa: F401
    import concourse.tile as tile
    from concourse import bacc, mybir

    f32 = mybir.dt.float32
    f32r = mybir.dt.float32r
    bf16 = mybir.dt.bfloat16

    nc = bacc.Bacc("TRN2", target_bir_lowering=False, debug=False,
                   enable_asserts=True, num_devices=N_CORES)

    xt = nc.dram_tensor("xt", [D, S], f32r, kind="ExternalInput")
    wq = nc.dram_tensor("wq", [D, LOCAL], f32r, kind="ExternalInput")
    wk = nc.dram_tensor("wk", [D, LOCAL], f32r, kind="ExternalInput")
    wv = nc.dram_tensor("wv", [D, LOCAL], f32r, kind="ExternalInput")
    wo = nc.dram_tensor("wo", [D, LOCAL], f32r, kind="ExternalInput")
    cost = nc.dram_tensor("cost", [ROT, S], bf16, kind="ExternalInput")
    sint = nc.dram_tensor("sint", [ROT, S], bf16, kind="ExternalInput")
    rt = nc.dram_tensor("rt", [ROT, ROT], f32r, kind="ExternalInput")
    ones = nc.dram_tensor("ones", [128, 1], f32r, kind="ExternalInput")
    onesr = nc.dram_tensor("onesr", [1, 128], f32r, kind="ExternalInput")
    masks = nc.dram_tensor("masks", [128, 4, 512], f32, kind="ExternalInput")
    y = nc.dram_tensor("y", [S, LOCAL], f32, kind="ExternalOutput")

    xt_r = xt.ap().rearrange("(dt p) s -> p dt s", p=128)
    wq_r = wq.ap().rearrange("(dt p) c -> p dt c", p=128)
    wk_r = wk.ap().rearrange("(dt p) c -> p dt c", p=128)
    wv_r = wv.ap().rearrange("(dt p) c -> p dt c", p=128)
    wo_r = wo.ap().rearrange("(dt p) c -> p dt c", p=128)

    with tile.TileContext(nc) as tc:
        with tc.tile_pool(name="dram", bufs=1, space="DRAM") as dpool, \
             tc.tile_pool(name="const", bufs=1) as cpool, \
             tc.tile_pool(name="psum", bufs=8, space="PSUM") as psp:
            qT_d = dpool.tile([LOCAL, S], f32r, name="qT_d")
            kT_d = dpool.tile([LOCAL, S], f32r, name="kT_d")
            v_d = dpool.tile([S, LOCAL], f32r, name="v_d")
            og_in = dpool.tile([LOCAL, S], f32r, name="og_in")
            og_out = dpool.tile([MP * LOCAL, S], f32r, name="og_out")

            rt_sb = cpool.tile([ROT, ROT], f32r, name="rt_sb")
            nc.sync.dma_start(rt_sb[:], rt.ap())
            ones_sb = cpool.tile([128, 1], f32r, name="ones_sb")
            nc.sync.dma_start(ones_sb[:], ones.ap())
            onesr_sb = cpool.tile([1, 128], f32r, name="onesr_sb")
            nc.sync.dma_start(onesr_sb[:], onesr.ap())

            tens = (xt_r, wq_r, wk_r, wv_r, wo_r, cost, sint, masks, y,
                    qT_d, kT_d, v_d, og_in, og_out, rt_sb, ones_sb, onesr_sb)
            for rep in range(n_repeat):
                _emit_body(nc, tc, tens, psp, cpool, with_collective, rep)

    nc.compile()
    return nc


def _rotary_tables(position_ids):
    """Transposed, interleave-repeated sin/cos tables: [64, S] per batch."""
    import ml_dtypes
    pos = np.asarray(position_ids).astype(np.int64)
    inv_freq = 1.0 / (10000.0 ** (np.arange(0, ROT, 2, dtype=np.float32) / ROT))
    sinusoid = np.arange(2048, dtype=np.float32)[:, None] * inv_freq[None, :]
    sin_t = np.sin(sinusoid).astype(np.float32)   # [2048, 32]
    cos_t = np.cos(sinusoid).astype(np.float32)
    outs = []
    for b in range(pos.shape[0]):
        sg = np.repeat(sin_t[pos[b]], 2, axis=1).T   # [64, S]
        cg = np.repeat(cos_t[pos[b]], 2, axis=1).T
        outs.append((np.ascontiguousarray(sg).astype(ml_dtypes.bfloat16),
                     np.ascontiguousarray(cg).astype(ml_dtypes.bfloat16)))
    return outs


def _consts():
    rt_np = np.zeros((ROT, ROT), dtype=np.float32)
    for i in range(ROT // 2):
        rt_np[2 * i + 1, 2 * i] = -1.0   # rt = R^T for rotate_every_two
        rt_np[2 * i, 2 * i + 1] = 1.0
    ones_np = np.ones((128, 1), dtype=np.float32)
    onesr_np = np.ones((1, 128), dtype=np.float32)
    masks_np = np.zeros((128, 4, 512), dtype=np.float32)
    ii = np.arange(128)[:, None]
    qq = np.arange(512)[None, :]
    for j in range(4):
        masks_np[:, j, :] = (128 * j + ii <= qq).astype(np.float32)
    return rt_np, onesr_np, ones_np, masks_np


def _in_maps(hidden_states, position_ids, W_qkv, W_out):
    hs = np.asarray(hidden_states, dtype=np.float32)
    wqkv = np.asarray(W_qkv, dtype=np.float32)
    wout = np.asarray(W_out, dtype=np.float32)
    rt_np, onesr_np, ones_np, masks_np = _consts()
    trig = _rotary_tables(position_ids)

    xts = [np.ascontiguousarray(hs[b].T) for b in range(B)]
    in_maps = []
    for c in range(N_CORES):
        dp, tp = c // MP, c % MP
        wl = wqkv[:, tp * 3 * LOCAL:(tp + 1) * 3 * LOCAL]
        sg, cg = trig[dp]
        # og_out is gathered chunk-major (rows (h*2+dm)*512 + tp2*128 + i);
        # permute W_out's contraction rows (tp2*1024+h*256+dm*128+i) to match.
        wo_c = wout[:, tp * LOCAL:(tp + 1) * LOCAL]
        wo_c = (wo_c.reshape(MP, H_LOC, 2, 128, LOCAL)
                .transpose(1, 2, 0, 3, 4).reshape(D, LOCAL))
        in_maps.append({
            "xt": xts[dp],
            "wq": np.ascontiguousarray(wl[:, 0:LOCAL]),
            "wv": np.ascontiguousarray(wl[:, LOCAL:2 * LOCAL]),
            "wk": np.ascontiguousarray(wl[:, 2 * LOCAL:3 * LOCAL]),
            "wo": np.ascontiguousarray(wo_c),
            "cost": cg, "sint": sg,
            "rt": rt_np, "ones": ones_np, "onesr": onesr_np,
            "masks": masks_np,
        })
    return in_maps


def _get_runner(n_repeat=1):
    key = ("runner", n_repeat)
    if key in _CACHE:
        return _CACHE[key]
    import jax
    from jax.sharding import Mesh, PartitionSpec, NamedSharding
    from jax.experimental.shard_map import shard_map
    from concourse import bass2jax, mybir

    nc = _build_program(with_collective=True, n_repeat=n_repeat)
    bass2jax.install_neuronx_cc_hook()

    partition_name = (nc.partition_id_tensor.name
                      if nc.partition_id_tensor else None)
    in_names, out_names, out_avals, zero_outs = [], [], [], []
    for alloc in nc.m.functions[0].allocations:
        if not isinstance(alloc, mybir.MemoryLocationSet):
            continue
        name = alloc.memorylocations[0].name
        if alloc.kind == "ExternalInput":
            if name != partition_name:
                in_names.append(name)
        elif alloc.kind == "ExternalOutput":
            shape = tuple(alloc.tensor_shape)
            dtype = mybir.dt.np(alloc.dtype)
            out_names.append(name)
            out_avals.append(jax.core.ShapedArray(shape, dtype))
            zero_outs.append(np.zeros(shape, dtype))
    n_params = len(in_names)
    all_names = in_names + out_names
    if partition_name is not None:
        all_names = all_names + [partition_name]

    def _body(*args):
        operands = list(args)
        if partition_name is not None:
            operands.append(bass2jax.partition_id_tensor())
        outs = bass2jax._bass_exec_p.bind(
            *operands,
            out_avals=tuple(out_avals),
            in_names=tuple(all_names),
            out_names=tuple(out_names),
            lowering_input_output_aliases=(),
            sim_require_finite=True,
            sim_require_nnan=True,
            nc=nc,
        )
        return tuple(outs)

    devices = jax.devices()[:N_CORES]
    mesh = Mesh(np.asarray(devices), ("core",))
    n_outs = len(out_names)
    sharded = jax.jit(
        shard_map(_body, mesh=mesh,
                  in_specs=(PartitionSpec("core"),) * (n_params + n_outs),
                  out_specs=(PartitionSpec("core"),) * n_outs,
                  check_rep=False),
        keep_unused=True,
    )
    sharding = NamedSharding(mesh, PartitionSpec("core"))
    runner = {
        "nc": nc, "sharded": sharded, "in_names": in_names,
        "out_names": out_names, "out_avals": out_avals,
        "zero_outs": zero_outs, "sharding": sharding, "jax": jax,
    }
    _CACHE[key] = runner
    return runner


def _stage(runner, in_maps):
    jax = runner["jax"]
    concat_in = [
        np.concatenate([np.asarray(in_maps[c][name]) for c in range(N_CORES)],
                       axis=0)
        for name in runner["in_names"]
    ]
    concat_zero = [
        np.zeros((N_CORES * z.shape[0], *z.shape[1:]), z.dtype)
        for z in runner["zero_outs"]
    ]
    return [jax.device_put(a, runner["sharding"]) for a in concat_in + concat_zero]


def _execute(runner, staged):
    jax = runner["jax"]
    outs = runner["sharded"](*staged)
    outs = jax.block_until_ready(outs)
    return outs


def kernel(hidden_states, position_ids, W_qkv, W_out):
    runner = _get_runner()
    in_maps = _in_maps(hidden_states, position_ids, W_qkv, W_out)
    staged = _stage(runner, in_maps)
    outs = _execute(runner, staged)
    yc = np.asarray(outs[0]).reshape(N_CORES, S, LOCAL)
    result = np.empty((B, S, D), dtype=np.float32)
    for c in range(N_CORES):
        dp, tp = c // MP, c % MP
        result[dp][:, tp * LOCAL:(tp + 1) * LOCAL] = yc[c]
    return result


def bench(inputs, iters=10, n_repeat=1):
    """Return per-call wall-clock seconds (list) for the staged executable."""
    import time
    runner = _get_runner(n_repeat)
    in_maps = _in_maps(**inputs)
    staged = _stage(runner, in_maps)
    _execute(runner, staged)  # warm-up / compile
    times = []
    for _ in range(iters):
        t0 = time.perf_counter()
        _execute(runner, staged)
        times.append(time.perf_counter() - t0)
    return times



# revision 14
# speedup vs baseline: 1.1431x; 1.1431x over previous
"""CodeGen attention on 8 Trainium2 NeuronCores (Bass/Tile).

Sharding: tensor-parallel over the 4 CodeGen mp head-groups x data-parallel
over batch 2. Core c = dp*4 + tp handles batch dp, head group tp (4 heads).

Per-core pipeline (all matmuls fp32r):
  phase 1: QKV projection. Q^T/K^T produced transposed (W stationary,
           X^T moving) with rotary fused on the first 64 rows of each head;
           V produced in natural [s, d] layout (X^T stationary, Wv moving).
  phase 2: causal attention per head. scores^T tiles, softmax without
           max-subtraction (logits are O(5)), column sums via an ones-vector
           matmul, PV accumulated in PSUM, late normalize by 1/rowsum
           broadcast through a K=1 matmul. The inner ki loop is software-
           pipelined: scores(ki+1) is emitted before rowsum/PV(ki) so the
           Tensor queue never stalls on the exp/mask chain.
  Per-head chunked AllGather (within each batch group of 4), each chunk
  issued as soon as its head finishes so 3/4 of the collective overlaps
  attention compute. og_out is head-major; the host permutes W_out rows
  to compensate.
  phase 3: out-projection against this core's 1024-column slice of W_out.

Host assembles the [B, S, D] output from per-core [S, D/4] column shards.
"""

import numpy as np

B, S, D = 2, 2048, 4096
N_HEAD = 16
HD = 256
MP = 4
ROT = 64
LOCAL = D // MP            # 1024 (= 4 heads * 256)
QK_TILES = 2 * LOCAL // 128  # 16: q col-tiles then k col-tiles
DT = D // 128              # 32 contraction tiles
N_CORES = 8
H_LOC = N_HEAD // MP       # 4 heads per core

_CACHE = {}


def _emit_body(nc, tc, tens, psp, cpool, with_collective, rep, phases=(1, 2, 3)):
    """One full pipeline pass (phases 1-3). rep only namespaces DRAM scratch."""
    import concourse.tile as tile  # noqa: F401
    from concourse import mybir

    f32 = mybir.dt.float32
    f32r = mybir.dt.float32r
    bf16 = mybir.dt.bfloat16
    EXP = mybir.ActivationFunctionType.Exp

    (xt_r, wq_r, wk_r, wv_r, wo_r, cost, sint, masks, y,
     qT_d, kT_d, v_d, og_in, og_out, rt_sb, ones_sb, onesr_sb,
     ones, warm_d) = tens

    # warm-up AllGather: absorbs cross-core start skew so the real
    # per-head chunks see a short rendezvous. (Collectives cannot read
    # IO tensors, so bounce `ones` through DRAM scratch first.)
    if with_collective and rep == 0:
        nc.sync.dma_start(warm_d[0:4, :], ones.ap()[0:4, :])
        nc.gpsimd.collective_compute(
            "AllGather", mybir.AluOpType.bypass,
            replica_groups=[[0, 1, 2, 3], [4, 5, 6, 7]],
            ins=[warm_d[0:4, :].opt()],
            outs=[warm_d[4:4 + 4 * MP, :].opt()],
        )

    # ---------------- phase 1: QKV ----------------
    if 1 in phases:
     with tc.tile_pool(name="xrp", bufs=1) as xrp, \
         tc.tile_pool(name="wmp", bufs=2) as wmp, \
         tc.tile_pool(name="sqp", bufs=3) as sqp, \
         tc.tile_pool(name="t1p", bufs=2) as t1p, \
         tc.tile_pool(name="t2p", bufs=2) as t2p, \
         tc.tile_pool(name="wvp", bufs=3) as wvp, \
         tc.tile_pool(name="trig", bufs=1) as trig:
        cost_sb = trig.tile([ROT, S], bf16, name="cost_sb")
        nc.sync.dma_start(cost_sb[:], cost.ap())
        sint_sb = trig.tile([ROT, S], bf16, name="sint_sb")
        nc.sync.dma_start(sint_sb[:], sint.ap())

        for ss in range(2):  # 1024-wide s slices
            xr = xrp.tile([128, DT, 1024], f32r, name="xr", tag="xr")
            for dt in range(DT):
                nc.sync.dma_start(
                    xr[:, dt, :], xt_r[:, dt, ss * 1024:(ss + 1) * 1024])

            # Q^T / K^T (W stationary, X^T moving), rotary fused
            for m in range(QK_TILES):
                w_r = wq_r if m < 8 else wk_r
                col0 = (m % 8) * 128
                wm = wmp.tile([128, DT, 128], f32r, name="wm", tag="wm")
                nc.sync.dma_start(wm[:], w_r[:, :, col0:col0 + 128])
                for n in range(2):
                    ps = psp.tile([128, 512], f32, name="ps", tag="ps")
                    for dt in range(DT):
                        nc.tensor.matmul(
                            ps[:], wm[:, dt, :],
                            xr[:, dt, n * 512:(n + 1) * 512],
                            start=(dt == 0), stop=(dt == DT - 1))
                    sg = ss * 1024 + n * 512
                    dest = qT_d if m < 8 else kT_d
                    sq = sqp.tile([128, 512], f32r, name="sq", tag="sq")
                    nc.vector.tensor_copy(sq[:], ps[:])
                    if m % 2 == 0:
                        # rows 0:64 are the rotary dims of a head
                        rp = psp.tile([128, 512], f32, name="rp", tag="ps")
                        nc.tensor.matmul(rp[0:ROT, :], rt_sb[:],
                                         sq[0:ROT, :], start=True, stop=True)
                        t1 = t1p.tile([ROT, 512], f32, name="t1", tag="t1")
                        nc.vector.tensor_mul(t1[:], ps[0:ROT, :],
                                             cost_sb[:, sg:sg + 512])
                        t2 = t2p.tile([ROT, 512], f32, name="t2", tag="t2")
                        nc.vector.tensor_mul(t2[:], rp[0:ROT, :],
                                             sint_sb[:, sg:sg + 512])
                        nc.vector.tensor_add(sq[0:ROT, :], t1[:], t2[:])
                    nc.sync.dma_start(
                        dest[col0:col0 + 128, sg:sg + 512], sq[:])

            # V natural layout (X^T stationary, Wv moving)
            for vn in range(2):
                pss = [psp.tile([128, 512], f32, name=f"vps{sm}", tag="ps")
                       for sm in range(8)]
                for dt in range(DT):
                    wvt = wvp.tile([128, 512], f32r, name="wvt", tag="wvt")
                    nc.sync.dma_start(
                        wvt[:], wv_r[:, dt, vn * 512:(vn + 1) * 512])
                    for sm in range(8):
                        nc.tensor.matmul(
                            pss[sm][:], xr[:, dt, sm * 128:(sm + 1) * 128],
                            wvt[:], start=(dt == 0), stop=(dt == DT - 1))
                for sm in range(8):
                    vc = sqp.tile([128, 512], f32r, name="vc", tag="sq")
                    nc.vector.tensor_copy(vc[:], pss[sm][:])
                    nc.sync.dma_start(
                        v_d[ss * 1024 + sm * 128:ss * 1024 + (sm + 1) * 128,
                            vn * 512:(vn + 1) * 512], vc[:])

    # ------- phase 2: attention, fused with chunked out-projection -------
    # Per-head AllGather chunks (bf16) fire as each head's output is
    # stored; the out-proj contribution of chunk c (contraction rows
    # c*1024..(c+1)*1024) is computed at the end of head c+1 and
    # accumulated into bf16 SBUF tiles, so only the last chunk's matmuls
    # + a vector add remain after attention ends.
    if 2 in phases:
     og_r = og_out[:].rearrange("(dt p) s -> p dt s", p=128)
     with tc.tile_pool(name="qtp", bufs=2) as qtp, \
         tc.tile_pool(name="ktp", bufs=2) as ktp, \
         tc.tile_pool(name="vtp", bufs=2) as vtp, \
         tc.tile_pool(name="etp", bufs=5) as etp, \
         tc.tile_pool(name="etmp", bufs=2) as etmp, \
         tc.tile_pool(name="otp", bufs=2) as otp, \
         tc.tile_pool(name="rbp", bufs=2) as rbp, \
         tc.tile_pool(name="rip", bufs=2) as rip, \
         tc.tile_pool(name="mkp", bufs=1) as mkp, \
         tc.tile_pool(name="accp", bufs=1) as accp, \
         tc.tile_pool(name="wocp", bufs=1) as wocp, \
         tc.tile_pool(name="omp3", bufs=3) as omp3, \
         tc.tile_pool(name="resp", bufs=4) as resp:
        masks_sb = mkp.tile([128, 4, 512], f32, name="masks_sb")
        nc.sync.dma_start(masks_sb[:], masks.ap())
        acc = accp.tile([128, 16, 2, 512], bf16, name="acc")

        def emit_partial_chunk(c):
            """Out-proj contribution of og chunk c (8 dt tiles)."""
            woc = wocp.tile([128, 8, LOCAL], bf16, name="woc", tag="woc")
            for j in range(8):
                nc.sync.dma_start(woc[:, j, :], wo_r[:, c * 8 + j, :])
            for qm in range(16):
                om = omp3.tile([128, 8, 128], bf16, name="om3", tag="om3")
                nc.sync.dma_start(
                    om[:], og_r[:, c * 8:(c + 1) * 8,
                                qm * 128:(qm + 1) * 128])
                for cn in range(2):
                    ps3 = psp.tile([128, 512], f32, name="ps3", tag="ps")
                    for j in range(8):
                        nc.tensor.matmul(
                            ps3[:], om[:, j, :],
                            woc[:, j, cn * 512:(cn + 1) * 512],
                            start=(j == 0), stop=(j == 7))
                    with nc.allow_low_precision(reason="bf16 partial acc"):
                        if c == 0:
                            nc.vector.tensor_copy(acc[:, qm, cn, :], ps3[:])
                        elif c < H_LOC - 1:
                            nc.vector.tensor_add(acc[:, qm, cn, :],
                                                 acc[:, qm, cn, :], ps3[:])
                        else:
                            res = resp.tile([128, 512], f32, name="res",
                                            tag="res")
                            nc.vector.tensor_add(res[:], acc[:, qm, cn, :],
                                                 ps3[:])
                            nc.sync.dma_start(
                                y.ap()[qm * 128:(qm + 1) * 128,
                                       cn * 512:(cn + 1) * 512], res[:])

        for h in range(H_LOC):
            r0 = h * HD
            qt = qtp.tile([128, 2, S], f32r, name="qt", tag="qt")
            qsrc = qT_d[r0:r0 + HD, :].rearrange("(dd p) s -> p dd s", p=128)
            for dd in range(2):
                nc.sync.dma_start(qt[:, dd, :], qsrc[:, dd, :])
            kt = ktp.tile([128, 2, S], f32r, name="kt", tag="kt")
            ksrc = kT_d[r0:r0 + HD, :].rearrange("(dd p) s -> p dd s", p=128)
            for dd in range(2):
                nc.sync.dma_start(kt[:, dd, :], ksrc[:, dd, :])
            vt = vtp.tile([128, S // 128, HD], f32r, name="vt", tag="vt")
            vsrc = v_d[:, r0:r0 + HD].rearrange("(st p) c -> p st c", p=128)
            for st in range(S // 128):
                nc.sync.dma_start(vt[:, st, :], vsrc[:, st, :])
            ot = otp.tile([128, 2, S], bf16, name="ot", tag="ot")

            for qn in range(4):
                nk = (qn + 1) * 4
                q0 = qn * 512
                rs = psp.tile([1, 512], f32, name="rs", tag="ps")
                ov = [psp.tile([128, 512], f32, name=f"ov{dm}", tag="ps")
                      for dm in range(2)]

                def _consume(et, ki):
                    nc.tensor.matmul(rs[:], ones_sb[:], et[:],
                                     start=(ki == 0), stop=(ki == nk - 1))
                    for dm in range(2):
                        nc.tensor.matmul(
                            ov[dm][:], vt[:, ki, dm * 128:(dm + 1) * 128],
                            et[:], start=(ki == 0), stop=(ki == nk - 1))

                # software pipeline: scores(ki) on Tensor overlap exp(ki-1)
                # on Scalar/Vector; PV(ki-1) follows scores(ki) in Tensor
                # queue order so the exp latency is hidden.
                prev = None
                for ki in range(nk):
                    sp = psp.tile([128, 512], f32, name="sp", tag="ps")
                    for dd in range(2):
                        nc.tensor.matmul(
                            sp[:], kt[:, dd, ki * 128:(ki + 1) * 128],
                            qt[:, dd, q0:q0 + 512],
                            start=(dd == 0), stop=(dd == 1))
                    if prev is not None:
                        _consume(*prev)
                    et = etp.tile([128, 512], f32r, name="et", tag="et")
                    if ki >= qn * 4:
                        etm = etmp.tile([128, 512], f32, name="etm", tag="etm")
                        nc.scalar.activation(etm[:], sp[:], EXP,
                                             bias=0.0, scale=1.0 / 16.0)
                        nc.vector.tensor_mul(et[:], etm[:],
                                             masks_sb[:, ki - qn * 4, :])
                    else:
                        nc.scalar.activation(et[:], sp[:], EXP,
                                             bias=0.0, scale=1.0 / 16.0)
                    prev = (et, ki)
                _consume(*prev)
                rinv = rip.tile([1, 512], f32r, name="rinv", tag="rinv")
                # f32r is bit-identical to f32 here; only the matmul
                # datapath reads it differently.
                with nc.allow_low_precision(reason="f32r == f32 bits"):
                    nc.vector.reciprocal(rinv[:], rs[:])
                rb = psp.tile([128, 512], f32, name="rb", tag="ps")
                nc.tensor.matmul(rb[:], onesr_sb[:], rinv[:],
                                 start=True, stop=True)
                rb_sb = rbp.tile([128, 512], f32, name="rb_sb", tag="rb")
                nc.vector.tensor_copy(rb_sb[:], rb[:])
                for dm in range(2):
                    nc.vector.tensor_mul(ot[:, dm, q0:q0 + 512],
                                         ov[dm][:], rb_sb[:])
            for dm in range(2):
                rr = (h * 2 + dm) * 128
                nc.sync.dma_start(og_in[rr:rr + 128, :], ot[:, dm, :])

            # per-head chunked AllGather (bf16), overlapped with the next
            # head's attention. og_out is head-major (rows h*1024 +
            # tp*256 + i); the host permutes W_out rows to match.
            if 3 in phases:
                if with_collective:
                    nc.gpsimd.collective_compute(
                        "AllGather",
                        mybir.AluOpType.bypass,
                        replica_groups=[[0, 1, 2, 3], [4, 5, 6, 7]],
                        ins=[og_in[h * HD:(h + 1) * HD, :].opt()],
                        outs=[og_out[h * MP * HD:(h + 1) * MP * HD, :].opt()],
                    )
                else:
                    for blk in range(MP):
                        nc.sync.dma_start(
                            og_out[h * MP * HD + blk * HD:
                                   h * MP * HD + (blk + 1) * HD, :],
                            og_in[h * HD:(h + 1) * HD, :])
                # chunk h-1's AllGather has had a full head of attention
                # to complete; fold its out-proj contribution in now.
                if h >= 1:
                    emit_partial_chunk(h - 1)

        if 3 in phases:
            emit_partial_chunk(H_LOC - 1)


def _build_program(with_collective=True, n_repeat=1):
    import concourse.bass as bass  # noqa: F401
    import concourse.tile as tile
    from concourse import bacc, mybir

    f32 = mybir.dt.float32
    f32r = mybir.dt.float32r
    bf16 = mybir.dt.bfloat16

    nc = bacc.Bacc("TRN2", target_bir_lowering=False, debug=False,
                   enable_asserts=True, num_devices=N_CORES)

    xt = nc.dram_tensor("xt", [D, S], f32r, kind="ExternalInput")
    wq = nc.dram_tensor("wq", [D, LOCAL], f32r, kind="ExternalInput")
    wk = nc.dram_tensor("wk", [D, LOCAL], f32r, kind="ExternalInput")
    wv = nc.dram_tensor("wv", [D, LOCAL], f32r, kind="ExternalInput")
    wo = nc.dram_tensor("wo", [D, LOCAL], bf16, kind="ExternalInput")
    cost = nc.dram_tensor("cost", [ROT, S], bf16, kind="ExternalInput")
    sint = nc.dram_tensor("sint", [ROT, S], bf16, kind="ExternalInput")
    rt = nc.dram_tensor("rt", [ROT, ROT], f32r, kind="ExternalInput")
    ones = nc.dram_tensor("ones", [128, 1], f32r, kind="ExternalInput")
    onesr = nc.dram_tensor("onesr", [1, 128], f32r, kind="ExternalInput")
    masks = nc.dram_tensor("masks", [128, 4, 512], f32, kind="ExternalInput")
    y = nc.dram_tensor("y", [S, LOCAL], f32, kind="ExternalOutput")

    xt_r = xt.ap().rearrange("(dt p) s -> p dt s", p=128)
    wq_r = wq.ap().rearrange("(dt p) c -> p dt c", p=128)
    wk_r = wk.ap().rearrange("(dt p) c -> p dt c", p=128)
    wv_r = wv.ap().rearrange("(dt p) c -> p dt c", p=128)
    wo_r = wo.ap().rearrange("(dt p) c -> p dt c", p=128)

    with tile.TileContext(nc) as tc:
        with tc.tile_pool(name="dram", bufs=1, space="DRAM") as dpool, \
             tc.tile_pool(name="const", bufs=1) as cpool, \
             tc.tile_pool(name="psum", bufs=8, space="PSUM") as psp:
            qT_d = dpool.tile([LOCAL, S], f32r, name="qT_d")
            kT_d = dpool.tile([LOCAL, S], f32r, name="kT_d")
            v_d = dpool.tile([S, LOCAL], f32r, name="v_d")
            og_in = dpool.tile([LOCAL, S], bf16, name="og_in")
            og_out = dpool.tile([MP * LOCAL, S], bf16, name="og_out")
            warm_d = dpool.tile([4 + 4 * MP, 1], f32r, name="warm_d")

            rt_sb = cpool.tile([ROT, ROT], f32r, name="rt_sb")
            nc.sync.dma_start(rt_sb[:], rt.ap())
            ones_sb = cpool.tile([128, 1], f32r, name="ones_sb")
            nc.sync.dma_start(ones_sb[:], ones.ap())
            onesr_sb = cpool.tile([1, 128], f32r, name="onesr_sb")
            nc.sync.dma_start(onesr_sb[:], onesr.ap())

            tens = (xt_r, wq_r, wk_r, wv_r, wo_r, cost, sint, masks, y,
                    qT_d, kT_d, v_d, og_in, og_out, rt_sb, ones_sb,
                    onesr_sb, ones, warm_d)
            for rep in range(n_repeat):
                _emit_body(nc, tc, tens, psp, cpool, with_collective, rep)

    nc.compile()
    return nc


def _rotary_tables(position_ids):
    """Transposed, interleave-repeated sin/cos tables: [64, S] per batch."""
    import ml_dtypes
    pos = np.asarray(position_ids).astype(np.int64)
    inv_freq = 1.0 / (10000.0 ** (np.arange(0, ROT, 2, dtype=np.float32) / ROT))
    sinusoid = np.arange(2048, dtype=np.float32)[:, None] * inv_freq[None, :]
    sin_t = np.sin(sinusoid).astype(np.float32)   # [2048, 32]
    cos_t = np.cos(sinusoid).astype(np.float32)
    outs = []
    for b in range(pos.shape[0]):
        sg = np.repeat(sin_t[pos[b]], 2, axis=1).T   # [64, S]
        cg = np.repeat(cos_t[pos[b]], 2, axis=1).T
        outs.append((np.ascontiguousarray(sg).astype(ml_dtypes.bfloat16),
                     np.ascontiguousarray(cg).astype(ml_dtypes.bfloat16)))
    return outs


def _consts():
    rt_np = np.zeros((ROT, ROT), dtype=np.float32)
    for i in range(ROT // 2):
        rt_np[2 * i + 1, 2 * i] = -1.0   # rt = R^T for rotate_every_two
        rt_np[2 * i, 2 * i + 1] = 1.0
    ones_np = np.ones((128, 1), dtype=np.float32)
    onesr_np = np.ones((1, 128), dtype=np.float32)
    masks_np = np.zeros((128, 4, 512), dtype=np.float32)
    ii = np.arange(128)[:, None]
    qq = np.arange(512)[None, :]
    for j in range(4):
        masks_np[:, j, :] = (128 * j + ii <= qq).astype(np.float32)
    return rt_np, onesr_np, ones_np, masks_np


def _in_maps(hidden_states, position_ids, W_qkv, W_out):
    import ml_dtypes
    hs = np.asarray(hidden_states, dtype=np.float32)
    wqkv = np.asarray(W_qkv, dtype=np.float32)
    wout = np.asarray(W_out, dtype=np.float32)
    rt_np, onesr_np, ones_np, masks_np = _consts()
    trig = _rotary_tables(position_ids)

    xts = [np.ascontiguousarray(hs[b].T) for b in range(B)]
    in_maps = []
    for c in range(N_CORES):
        dp, tp = c // MP, c % MP
        wl = wqkv[:, tp * 3 * LOCAL:(tp + 1) * 3 * LOCAL]
        sg, cg = trig[dp]
        # og_out is gathered head-major (rows h*1024 + tp2*256 + i); permute
        # W_out's contraction rows (tp2*1024 + h*256 + i) to match, in bf16.
        wo_c = wout[:, tp * LOCAL:(tp + 1) * LOCAL]
        wo_c = (wo_c.reshape(MP, H_LOC, HD, LOCAL)
                .transpose(1, 0, 2, 3).reshape(D, LOCAL)
                .astype(ml_dtypes.bfloat16))
        in_maps.append({
            "xt": xts[dp],
            "wq": np.ascontiguousarray(wl[:, 0:LOCAL]),
            "wv": np.ascontiguousarray(wl[:, LOCAL:2 * LOCAL]),
            "wk": np.ascontiguousarray(wl[:, 2 * LOCAL:3 * LOCAL]),
            "wo": np.ascontiguousarray(wo_c),
            "cost": cg, "sint": sg,
            "rt": rt_np, "ones": ones_np, "onesr": onesr_np,
            "masks": masks_np,
        })
    return in_maps


def _get_runner(n_repeat=1):
    key = ("runner", n_repeat)
    if key in _CACHE:
        return _CACHE[key]
    import jax
    from jax.sharding import Mesh, PartitionSpec, NamedSharding
    from jax.experimental.shard_map import shard_map
    from concourse import bass2jax, mybir

    nc = _build_program(with_collective=True, n_repeat=n_repeat)
    bass2jax.install_neuronx_cc_hook()

    partition_name = (nc.partition_id_tensor.name
                      if nc.partition_id_tensor else None)
    in_names, out_names, out_avals, zero_outs = [], [], [], []
    for alloc in nc.m.functions[0].allocations:
        if not isinstance(alloc, mybir.MemoryLocationSet):
            continue
        name = alloc.memorylocations[0].name
        if alloc.kind == "ExternalInput":
            if name != partition_name:
                in_names.append(name)
        elif alloc.kind == "ExternalOutput":
            shape = tuple(alloc.tensor_shape)
            dtype = mybir.dt.np(alloc.dtype)
            out_names.append(name)
            out_avals.append(jax.core.ShapedArray(shape, dtype))
            zero_outs.append(np.zeros(shape, dtype))
    n_params = len(in_names)
    all_names = in_names + out_names
    if partition_name is not None:
        all_names = all_names + [partition_name]

    def _body(*args):
        operands = list(args)
        if partition_name is not None:
            operands.append(bass2jax.partition_id_tensor())
        outs = bass2jax._bass_exec_p.bind(
            *operands,
            out_avals=tuple(out_avals),
            in_names=tuple(all_names),
            out_names=tuple(out_names),
            lowering_input_output_aliases=(),
            sim_require_finite=True,
            sim_require_nnan=True,
            nc=nc,
        )
        return tuple(outs)

    devices = jax.devices()[:N_CORES]
    mesh = Mesh(np.asarray(devices), ("core",))
    n_outs = len(out_names)
    sharded = jax.jit(
        shard_map(_body, mesh=mesh,
                  in_specs=(PartitionSpec("core"),) * (n_params + n_outs),
                  out_specs=(PartitionSpec("core"),) * n_outs,
                  check_rep=False),
        keep_unused=True,
    )
    sharding = NamedSharding(mesh, PartitionSpec("core"))
    runner = {
        "nc": nc, "sharded": sharded, "in_names": in_names,
        "out_names": out_names, "out_avals": out_avals,
        "zero_outs": zero_outs, "sharding": sharding, "jax": jax,
    }
    _CACHE[key] = runner
    return runner


def _stage(runner, in_maps):
    jax = runner["jax"]
    concat_in = [
        np.concatenate([np.asarray(in_maps[c][name]) for c in range(N_CORES)],
                       axis=0)
        for name in runner["in_names"]
    ]
    concat_zero = [
        np.zeros((N_CORES * z.shape[0], *z.shape[1:]), z.dtype)
        for z in runner["zero_outs"]
    ]
    return [jax.device_put(a, runner["sharding"]) for a in concat_in + concat_zero]


def _execute(runner, staged):
    jax = runner["jax"]
    outs = runner["sharded"](*staged)
    outs = jax.block_until_ready(outs)
    return outs


def kernel(hidden_states, position_ids, W_qkv, W_out):
    runner = _get_runner()
    in_maps = _in_maps(hidden_states, position_ids, W_qkv, W_out)
    staged = _stage(runner, in_maps)
    outs = _execute(runner, staged)
    yc = np.asarray(outs[0]).reshape(N_CORES, S, LOCAL)
    result = np.empty((B, S, D), dtype=np.float32)
    for c in range(N_CORES):
        dp, tp = c // MP, c % MP
        result[dp][:, tp * LOCAL:(tp + 1) * LOCAL] = yc[c]
    return result


def bench(inputs, iters=10, n_repeat=1):
    """Return per-call wall-clock seconds (list) for the staged executable."""
    import time
    runner = _get_runner(n_repeat)
    in_maps = _in_maps(**inputs)
    staged = _stage(runner, in_maps)
    _execute(runner, staged)  # warm-up / compile
    times = []
    for _ in range(iters):
        t0 = time.perf_counter()
        _execute(runner, staged)
        times.append(time.perf_counter() - t0)
    return times



# revision 16
# speedup vs baseline: 1.1544x; 1.0099x over previous
"""CodeGen attention on 8 Trainium2 NeuronCores (Bass/Tile).

Sharding: tensor-parallel over the 4 CodeGen mp head-groups x data-parallel
over batch 2. Core c = dp*4 + tp handles batch dp, head group tp (4 heads).

Per-core pipeline (all matmuls fp32r):
  phase 1: QKV projection. Q^T/K^T produced transposed (W stationary,
           X^T moving) with rotary fused on the first 64 rows of each head;
           V produced in natural [s, d] layout (X^T stationary, Wv moving).
  phase 2: causal attention per head. scores^T tiles, softmax without
           max-subtraction (logits are O(5)), column sums via an ones-vector
           matmul, PV accumulated in PSUM, late normalize by 1/rowsum
           broadcast through a K=1 matmul. The inner ki loop is software-
           pipelined: scores(ki+1) is emitted before rowsum/PV(ki) so the
           Tensor queue never stalls on the exp/mask chain.
  Per-head chunked AllGather (within each batch group of 4), each chunk
  issued as soon as its head finishes so 3/4 of the collective overlaps
  attention compute. og_out is head-major; the host permutes W_out rows
  to compensate.
  phase 3: out-projection against this core's 1024-column slice of W_out.

Host assembles the [B, S, D] output from per-core [S, D/4] column shards.
"""

import numpy as np

B, S, D = 2, 2048, 4096
N_HEAD = 16
HD = 256
MP = 4
ROT = 64
LOCAL = D // MP            # 1024 (= 4 heads * 256)
QK_TILES = 2 * LOCAL // 128  # 16: q col-tiles then k col-tiles
DT = D // 128              # 32 contraction tiles
N_CORES = 8
H_LOC = N_HEAD // MP       # 4 heads per core

_CACHE = {}


def _emit_body(nc, tc, tens, psp, cpool, with_collective, rep, phases=(1, 2, 3)):
    """One full pipeline pass (phases 1-3). rep only namespaces DRAM scratch."""
    import concourse.tile as tile  # noqa: F401
    from concourse import mybir

    f32 = mybir.dt.float32
    f32r = mybir.dt.float32r
    bf16 = mybir.dt.bfloat16
    EXP = mybir.ActivationFunctionType.Exp

    (xt_r, wq_r, wk_r, wv_r, wo_r, cost, sint, masks, y,
     qT_d, kT_d, v_d, og_in, og_out, rt_sb, ones_sb, onesr_sb,
     ones, warm_d, og_in3, og_out3) = tens

    # warm-up AllGather: absorbs cross-core start skew so the real
    # per-head chunks see a short rendezvous. (Collectives cannot read
    # IO tensors, so bounce `ones` through DRAM scratch first.)
    if with_collective and rep == 0:
        nc.sync.dma_start(warm_d[0:4, :], ones.ap()[0:4, :])
        nc.gpsimd.collective_compute(
            "AllGather", mybir.AluOpType.bypass,
            replica_groups=[[0, 1, 2, 3], [4, 5, 6, 7]],
            ins=[warm_d[0:4, :].opt()],
            outs=[warm_d[4:4 + 4 * MP, :].opt()],
        )

    # ---------------- phase 1: QKV ----------------
    if 1 in phases:
     with tc.tile_pool(name="xrp", bufs=1) as xrp, \
         tc.tile_pool(name="wmp", bufs=2) as wmp, \
         tc.tile_pool(name="sqp", bufs=3) as sqp, \
         tc.tile_pool(name="t1p", bufs=2) as t1p, \
         tc.tile_pool(name="t2p", bufs=2) as t2p, \
         tc.tile_pool(name="wvp", bufs=3) as wvp, \
         tc.tile_pool(name="trig", bufs=1) as trig:
        cost_sb = trig.tile([ROT, S], bf16, name="cost_sb")
        nc.sync.dma_start(cost_sb[:], cost.ap())
        sint_sb = trig.tile([ROT, S], bf16, name="sint_sb")
        nc.sync.dma_start(sint_sb[:], sint.ap())

        for ss in range(2):  # 1024-wide s slices
            xr = xrp.tile([128, DT, 1024], f32r, name="xr", tag="xr")
            for dt in range(DT):
                nc.sync.dma_start(
                    xr[:, dt, :], xt_r[:, dt, ss * 1024:(ss + 1) * 1024])

            # Q^T / K^T (W stationary, X^T moving), rotary fused.
            # Interleave q/k col-tiles per head so head h's attention
            # inputs are complete ~(h+1)/4 through each QK sweep.
            for m in [0, 1, 8, 9, 2, 3, 10, 11,
                      4, 5, 12, 13, 6, 7, 14, 15]:
                w_r = wq_r if m < 8 else wk_r
                col0 = (m % 8) * 128
                wm = wmp.tile([128, DT, 128], f32r, name="wm", tag="wm")
                nc.sync.dma_start(wm[:], w_r[:, :, col0:col0 + 128])
                for n in range(2):
                    ps = psp.tile([128, 512], f32, name="ps", tag="ps")
                    for dt in range(DT):
                        nc.tensor.matmul(
                            ps[:], wm[:, dt, :],
                            xr[:, dt, n * 512:(n + 1) * 512],
                            start=(dt == 0), stop=(dt == DT - 1))
                    sg = ss * 1024 + n * 512
                    dest = qT_d if m < 8 else kT_d
                    sq = sqp.tile([128, 512], f32r, name="sq", tag="sq")
                    nc.vector.tensor_copy(sq[:], ps[:])
                    if m % 2 == 0:
                        # rows 0:64 are the rotary dims of a head
                        rp = psp.tile([128, 512], f32, name="rp", tag="ps")
                        nc.tensor.matmul(rp[0:ROT, :], rt_sb[:],
                                         sq[0:ROT, :], start=True, stop=True)
                        t1 = t1p.tile([ROT, 512], f32, name="t1", tag="t1")
                        nc.vector.tensor_mul(t1[:], ps[0:ROT, :],
                                             cost_sb[:, sg:sg + 512])
                        t2 = t2p.tile([ROT, 512], f32, name="t2", tag="t2")
                        nc.vector.tensor_mul(t2[:], rp[0:ROT, :],
                                             sint_sb[:, sg:sg + 512])
                        nc.vector.tensor_add(sq[0:ROT, :], t1[:], t2[:])
                    nc.sync.dma_start(
                        dest[col0:col0 + 128, sg:sg + 512], sq[:])

            # V natural layout (X^T stationary, Wv moving)
            for vn in range(2):
                pss = [psp.tile([128, 512], f32, name=f"vps{sm}", tag="ps")
                       for sm in range(8)]
                for dt in range(DT):
                    wvt = wvp.tile([128, 512], f32r, name="wvt", tag="wvt")
                    nc.sync.dma_start(
                        wvt[:], wv_r[:, dt, vn * 512:(vn + 1) * 512])
                    for sm in range(8):
                        nc.tensor.matmul(
                            pss[sm][:], xr[:, dt, sm * 128:(sm + 1) * 128],
                            wvt[:], start=(dt == 0), stop=(dt == DT - 1))
                for sm in range(8):
                    vc = sqp.tile([128, 512], f32r, name="vc", tag="sq")
                    nc.vector.tensor_copy(vc[:], pss[sm][:])
                    nc.sync.dma_start(
                        v_d[ss * 1024 + sm * 128:ss * 1024 + (sm + 1) * 128,
                            vn * 512:(vn + 1) * 512], vc[:])

    # ------- phase 2: attention, fused with chunked out-projection -------
    # Per-head AllGather chunks (bf16) fire as each head's output is
    # stored; the out-proj contribution of chunk c (contraction rows
    # c*1024..(c+1)*1024) is computed at the end of head c+1 and
    # accumulated into bf16 SBUF tiles, so only the last chunk's matmuls
    # + a vector add remain after attention ends.
    if 2 in phases:
     og_r = og_out[:].rearrange("(dt p) s -> p dt s", p=128)
     with tc.tile_pool(name="qtp", bufs=2) as qtp, \
         tc.tile_pool(name="ktp", bufs=2) as ktp, \
         tc.tile_pool(name="vtp", bufs=2) as vtp, \
         tc.tile_pool(name="etp", bufs=5) as etp, \
         tc.tile_pool(name="etmp", bufs=2) as etmp, \
         tc.tile_pool(name="otp", bufs=2) as otp, \
         tc.tile_pool(name="rbp", bufs=2) as rbp, \
         tc.tile_pool(name="rip", bufs=2) as rip, \
         tc.tile_pool(name="mkp", bufs=1) as mkp, \
         tc.tile_pool(name="accp", bufs=1) as accp, \
         tc.tile_pool(name="wocp", bufs=1) as wocp, \
         tc.tile_pool(name="omp3", bufs=3) as omp3, \
         tc.tile_pool(name="resp", bufs=4) as resp:
        masks_sb = mkp.tile([128, 4, 512], f32, name="masks_sb")
        nc.sync.dma_start(masks_sb[:], masks.ap())
        acc = accp.tile([128, 16, 2, 512], bf16, name="acc")

        def emit_partial_chunk(c):
            """Out-proj contribution of og chunk c (8 dt tiles)."""
            woc = wocp.tile([128, 8, LOCAL], bf16, name="woc", tag="woc")
            for j in range(8):
                nc.sync.dma_start(woc[:, j, :], wo_r[:, c * 8 + j, :])
            for qm in range(16):
                om = omp3.tile([128, 8, 128], bf16, name="om3", tag="om3")
                nc.sync.dma_start(
                    om[:], og_r[:, c * 8:(c + 1) * 8,
                                qm * 128:(qm + 1) * 128])
                for cn in range(2):
                    ps3 = psp.tile([128, 512], f32, name="ps3", tag="ps")
                    for j in range(8):
                        nc.tensor.matmul(
                            ps3[:], om[:, j, :],
                            woc[:, j, cn * 512:(cn + 1) * 512],
                            start=(j == 0), stop=(j == 7))
                    with nc.allow_low_precision(reason="bf16 partial acc"):
                        if c == 0:
                            nc.vector.tensor_copy(acc[:, qm, cn, :], ps3[:])
                        elif c < H_LOC - 1:
                            nc.vector.tensor_add(acc[:, qm, cn, :],
                                                 acc[:, qm, cn, :], ps3[:])
                        else:
                            res = resp.tile([128, 512], f32, name="res",
                                            tag="res")
                            nc.vector.tensor_add(res[:], acc[:, qm, cn, :],
                                                 ps3[:])
                            nc.sync.dma_start(
                                y.ap()[qm * 128:(qm + 1) * 128,
                                       cn * 512:(cn + 1) * 512], res[:])

        for h in range(H_LOC):
            r0 = h * HD
            qt = qtp.tile([128, 2, S], f32r, name="qt", tag="qt")
            qsrc = qT_d[r0:r0 + HD, :].rearrange("(dd p) s -> p dd s", p=128)
            for dd in range(2):
                nc.sync.dma_start(qt[:, dd, :], qsrc[:, dd, :])
            kt = ktp.tile([128, 2, S], f32r, name="kt", tag="kt")
            ksrc = kT_d[r0:r0 + HD, :].rearrange("(dd p) s -> p dd s", p=128)
            for dd in range(2):
                nc.sync.dma_start(kt[:, dd, :], ksrc[:, dd, :])
            vt = vtp.tile([128, S // 128, HD], f32r, name="vt", tag="vt")
            vsrc = v_d[:, r0:r0 + HD].rearrange("(st p) c -> p st c", p=128)
            for st in range(S // 128):
                nc.sync.dma_start(vt[:, st, :], vsrc[:, st, :])
            ot = otp.tile([128, 2, S], bf16, name="ot", tag="ot")

            for qn in range(4):
                nk = (qn + 1) * 4
                q0 = qn * 512
                rs = psp.tile([1, 512], f32, name="rs", tag="ps")
                ov = [psp.tile([128, 512], f32, name=f"ov{dm}", tag="ps")
                      for dm in range(2)]

                def _consume(et, ki):
                    nc.tensor.matmul(rs[:], ones_sb[:], et[:],
                                     start=(ki == 0), stop=(ki == nk - 1))
                    for dm in range(2):
                        nc.tensor.matmul(
                            ov[dm][:], vt[:, ki, dm * 128:(dm + 1) * 128],
                            et[:], start=(ki == 0), stop=(ki == nk - 1))

                # software pipeline: scores(ki) on Tensor overlap exp(ki-1)
                # on Scalar/Vector; PV(ki-1) follows scores(ki) in Tensor
                # queue order so the exp latency is hidden.
                prev = None
                for ki in range(nk):
                    sp = psp.tile([128, 512], f32, name="sp", tag="ps")
                    for dd in range(2):
                        nc.tensor.matmul(
                            sp[:], kt[:, dd, ki * 128:(ki + 1) * 128],
                            qt[:, dd, q0:q0 + 512],
                            start=(dd == 0), stop=(dd == 1))
                    if prev is not None:
                        _consume(*prev)
                    et = etp.tile([128, 512], f32r, name="et", tag="et")
                    if ki >= qn * 4:
                        etm = etmp.tile([128, 512], f32, name="etm", tag="etm")
                        nc.scalar.activation(etm[:], sp[:], EXP,
                                             bias=0.0, scale=1.0 / 16.0)
                        nc.vector.tensor_mul(et[:], etm[:],
                                             masks_sb[:, ki - qn * 4, :])
                    else:
                        nc.scalar.activation(et[:], sp[:], EXP,
                                             bias=0.0, scale=1.0 / 16.0)
                    prev = (et, ki)
                _consume(*prev)
                rinv = rip.tile([1, 512], f32r, name="rinv", tag="rinv")
                # f32r is bit-identical to f32 here; only the matmul
                # datapath reads it differently.
                with nc.allow_low_precision(reason="f32r == f32 bits"):
                    nc.vector.reciprocal(rinv[:], rs[:])
                rb = psp.tile([128, 512], f32, name="rb", tag="ps")
                nc.tensor.matmul(rb[:], onesr_sb[:], rinv[:],
                                 start=True, stop=True)
                rb_sb = rbp.tile([128, 512], f32, name="rb_sb", tag="rb")
                nc.vector.tensor_copy(rb_sb[:], rb[:])
                for dm in range(2):
                    nc.vector.tensor_mul(ot[:, dm, q0:q0 + 512],
                                         ov[dm][:], rb_sb[:])
                # last head: gather each 512-query column block as soon as
                # it is normalized, so only the final 512 queries' gather
                # sits on the critical path after attention ends.
                if h == H_LOC - 1 and 3 in phases:
                    for dm in range(2):
                        nc.sync.dma_start(
                            og_in3[qn, dm * 128:(dm + 1) * 128, :],
                            ot[:, dm, q0:q0 + 512])
                    if with_collective:
                        nc.gpsimd.collective_compute(
                            "AllGather", mybir.AluOpType.bypass,
                            replica_groups=[[0, 1, 2, 3], [4, 5, 6, 7]],
                            ins=[og_in3[qn, :, :].opt()],
                            outs=[og_out3[qn, :, :].opt()],
                        )
                    else:
                        for blk in range(MP):
                            nc.sync.dma_start(
                                og_out3[qn, blk * 256:(blk + 1) * 256, :],
                                og_in3[qn, :, :])
            if h == H_LOC - 1:
                if 3 in phases:
                    emit_partial_chunk(h - 1)
                continue
            for dm in range(2):
                rr = (h * 2 + dm) * 128
                nc.sync.dma_start(og_in[rr:rr + 128, :], ot[:, dm, :])

            # per-head chunked AllGather (bf16), overlapped with the next
            # head's attention. og_out is head-major (rows h*1024 +
            # tp*256 + i); the host permutes W_out rows to match.
            if 3 in phases:
                if with_collective:
                    nc.gpsimd.collective_compute(
                        "AllGather",
                        mybir.AluOpType.bypass,
                        replica_groups=[[0, 1, 2, 3], [4, 5, 6, 7]],
                        ins=[og_in[h * HD:(h + 1) * HD, :].opt()],
                        outs=[og_out[h * MP * HD:(h + 1) * MP * HD, :].opt()],
                    )
                else:
                    for blk in range(MP):
                        nc.sync.dma_start(
                            og_out[h * MP * HD + blk * HD:
                                   h * MP * HD + (blk + 1) * HD, :],
                            og_in[h * HD:(h + 1) * HD, :])
                # chunk h-1's AllGather has had a full head of attention
                # to complete; fold its out-proj contribution in now.
                if h >= 1:
                    emit_partial_chunk(h - 1)

        if 3 in phases:
            # tail: last head's out-proj per 512-query column block; block
            # qn only needs its own small gather + the shared weights.
            woc3 = wocp.tile([128, 8, LOCAL], bf16, name="woc", tag="woc")
            for j in range(8):
                nc.sync.dma_start(
                    woc3[:, j, :], wo_r[:, (H_LOC - 1) * 8 + j, :])
            for qn in range(4):
                og3_r = og_out3[qn].rearrange("(j p) c -> p j c", p=128)
                for cq in range(4):
                    qm = qn * 4 + cq
                    om = omp3.tile([128, 8, 128], bf16, name="om3",
                                   tag="om3")
                    nc.sync.dma_start(
                        om[:], og3_r[:, :, cq * 128:(cq + 1) * 128])
                    for cn in range(2):
                        ps3 = psp.tile([128, 512], f32, name="ps3",
                                       tag="ps")
                        for j in range(8):
                            nc.tensor.matmul(
                                ps3[:], om[:, j, :],
                                woc3[:, j, cn * 512:(cn + 1) * 512],
                                start=(j == 0), stop=(j == 7))
                        res = resp.tile([128, 512], f32, name="res",
                                        tag="res")
                        with nc.allow_low_precision(reason="bf16 acc"):
                            nc.vector.tensor_add(
                                res[:], acc[:, qm, cn, :], ps3[:])
                        nc.sync.dma_start(
                            y.ap()[qm * 128:(qm + 1) * 128,
                                   cn * 512:(cn + 1) * 512], res[:])


def _build_program(with_collective=True, n_repeat=1):
    import concourse.bass as bass  # noqa: F401
    import concourse.tile as tile
    from concourse import bacc, mybir

    f32 = mybir.dt.float32
    f32r = mybir.dt.float32r
    bf16 = mybir.dt.bfloat16

    nc = bacc.Bacc("TRN2", target_bir_lowering=False, debug=False,
                   enable_asserts=True, num_devices=N_CORES)

    xt = nc.dram_tensor("xt", [D, S], f32r, kind="ExternalInput")
    wq = nc.dram_tensor("wq", [D, LOCAL], f32r, kind="ExternalInput")
    wk = nc.dram_tensor("wk", [D, LOCAL], f32r, kind="ExternalInput")
    wv = nc.dram_tensor("wv", [D, LOCAL], f32r, kind="ExternalInput")
    wo = nc.dram_tensor("wo", [D, LOCAL], bf16, kind="ExternalInput")
    cost = nc.dram_tensor("cost", [ROT, S], bf16, kind="ExternalInput")
    sint = nc.dram_tensor("sint", [ROT, S], bf16, kind="ExternalInput")
    rt = nc.dram_tensor("rt", [ROT, ROT], f32r, kind="ExternalInput")
    ones = nc.dram_tensor("ones", [128, 1], f32r, kind="ExternalInput")
    onesr = nc.dram_tensor("onesr", [1, 128], f32r, kind="ExternalInput")
    masks = nc.dram_tensor("masks", [128, 4, 512], f32, kind="ExternalInput")
    y = nc.dram_tensor("y", [S, LOCAL], f32, kind="ExternalOutput")

    xt_r = xt.ap().rearrange("(dt p) s -> p dt s", p=128)
    wq_r = wq.ap().rearrange("(dt p) c -> p dt c", p=128)
    wk_r = wk.ap().rearrange("(dt p) c -> p dt c", p=128)
    wv_r = wv.ap().rearrange("(dt p) c -> p dt c", p=128)
    wo_r = wo.ap().rearrange("(dt p) c -> p dt c", p=128)

    with tile.TileContext(nc) as tc:
        with tc.tile_pool(name="dram", bufs=1, space="DRAM") as dpool, \
             tc.tile_pool(name="const", bufs=1) as cpool, \
             tc.tile_pool(name="psum", bufs=8, space="PSUM") as psp:
            qT_d = dpool.tile([LOCAL, S], f32r, name="qT_d")
            kT_d = dpool.tile([LOCAL, S], f32r, name="kT_d")
            v_d = dpool.tile([S, LOCAL], f32r, name="v_d")
            og_in = dpool.tile([LOCAL, S], bf16, name="og_in")
            og_out = dpool.tile([(H_LOC - 1) * MP * HD, S], bf16,
                                name="og_out")
            og_in3 = dpool.tile([4, 2 * 128, 512], bf16, name="og_in3")
            og_out3 = dpool.tile([4, MP * 2 * 128, 512], bf16,
                                 name="og_out3")
            warm_d = dpool.tile([4 + 4 * MP, 1], f32r, name="warm_d")

            rt_sb = cpool.tile([ROT, ROT], f32r, name="rt_sb")
            nc.sync.dma_start(rt_sb[:], rt.ap())
            ones_sb = cpool.tile([128, 1], f32r, name="ones_sb")
            nc.sync.dma_start(ones_sb[:], ones.ap())
            onesr_sb = cpool.tile([1, 128], f32r, name="onesr_sb")
            nc.sync.dma_start(onesr_sb[:], onesr.ap())

            tens = (xt_r, wq_r, wk_r, wv_r, wo_r, cost, sint, masks, y,
                    qT_d, kT_d, v_d, og_in, og_out, rt_sb, ones_sb,
                    onesr_sb, ones, warm_d, og_in3, og_out3)
            for rep in range(n_repeat):
                _emit_body(nc, tc, tens, psp, cpool, with_collective, rep)

    nc.compile()
    return nc


def _rotary_tables(position_ids):
    """Transposed, interleave-repeated sin/cos tables: [64, S] per batch."""
    import ml_dtypes
    pos = np.asarray(position_ids).astype(np.int64)
    inv_freq = 1.0 / (10000.0 ** (np.arange(0, ROT, 2, dtype=np.float32) / ROT))
    sinusoid = np.arange(2048, dtype=np.float32)[:, None] * inv_freq[None, :]
    sin_t = np.sin(sinusoid).astype(np.float32)   # [2048, 32]
    cos_t = np.cos(sinusoid).astype(np.float32)
    outs = []
    for b in range(pos.shape[0]):
        sg = np.repeat(sin_t[pos[b]], 2, axis=1).T   # [64, S]
        cg = np.repeat(cos_t[pos[b]], 2, axis=1).T
        outs.append((np.ascontiguousarray(sg).astype(ml_dtypes.bfloat16),
                     np.ascontiguousarray(cg).astype(ml_dtypes.bfloat16)))
    return outs


def _consts():
    rt_np = np.zeros((ROT, ROT), dtype=np.float32)
    for i in range(ROT // 2):
        rt_np[2 * i + 1, 2 * i] = -1.0   # rt = R^T for rotate_every_two
        rt_np[2 * i, 2 * i + 1] = 1.0
    ones_np = np.ones((128, 1), dtype=np.float32)
    onesr_np = np.ones((1, 128), dtype=np.float32)
    masks_np = np.zeros((128, 4, 512), dtype=np.float32)
    ii = np.arange(128)[:, None]
    qq = np.arange(512)[None, :]
    for j in range(4):
        masks_np[:, j, :] = (128 * j + ii <= qq).astype(np.float32)
    return rt_np, onesr_np, ones_np, masks_np


def _in_maps(hidden_states, position_ids, W_qkv, W_out):
    import ml_dtypes
    hs = np.asarray(hidden_states, dtype=np.float32)
    wqkv = np.asarray(W_qkv, dtype=np.float32)
    wout = np.asarray(W_out, dtype=np.float32)
    rt_np, onesr_np, ones_np, masks_np = _consts()
    trig = _rotary_tables(position_ids)

    xts = [np.ascontiguousarray(hs[b].T) for b in range(B)]
    in_maps = []
    for c in range(N_CORES):
        dp, tp = c // MP, c % MP
        wl = wqkv[:, tp * 3 * LOCAL:(tp + 1) * 3 * LOCAL]
        sg, cg = trig[dp]
        # og_out is gathered head-major (rows h*1024 + tp2*256 + i); permute
        # W_out's contraction rows (tp2*1024 + h*256 + i) to match, in bf16.
        wo_c = wout[:, tp * LOCAL:(tp + 1) * LOCAL]
        wo_c = (wo_c.reshape(MP, H_LOC, HD, LOCAL)
                .transpose(1, 0, 2, 3).reshape(D, LOCAL)
                .astype(ml_dtypes.bfloat16))
        in_maps.append({
            "xt": xts[dp],
            "wq": np.ascontiguousarray(wl[:, 0:LOCAL]),
            "wv": np.ascontiguousarray(wl[:, LOCAL:2 * LOCAL]),
            "wk": np.ascontiguousarray(wl[:, 2 * LOCAL:3 * LOCAL]),
            "wo": np.ascontiguousarray(wo_c),
            "cost": cg, "sint": sg,
            "rt": rt_np, "ones": ones_np, "onesr": onesr_np,
            "masks": masks_np,
        })
    return in_maps


def _get_runner(n_repeat=1):
    key = ("runner", n_repeat)
    if key in _CACHE:
        return _CACHE[key]
    import jax
    from jax.sharding import Mesh, PartitionSpec, NamedSharding
    from jax.experimental.shard_map import shard_map
    from concourse import bass2jax, mybir

    nc = _build_program(with_collective=True, n_repeat=n_repeat)
    bass2jax.install_neuronx_cc_hook()

    partition_name = (nc.partition_id_tensor.name
                      if nc.partition_id_tensor else None)
    in_names, out_names, out_avals, zero_outs = [], [], [], []
    for alloc in nc.m.functions[0].allocations:
        if not isinstance(alloc, mybir.MemoryLocationSet):
            continue
        name = alloc.memorylocations[0].name
        if alloc.kind == "ExternalInput":
            if name != partition_name:
                in_names.append(name)
        elif alloc.kind == "ExternalOutput":
            shape = tuple(alloc.tensor_shape)
            dtype = mybir.dt.np(alloc.dtype)
            out_names.append(name)
            out_avals.append(jax.core.ShapedArray(shape, dtype))
            zero_outs.append(np.zeros(shape, dtype))
    n_params = len(in_names)
    all_names = in_names + out_names
    if partition_name is not None:
        all_names = all_names + [partition_name]

    def _body(*args):
        operands = list(args)
        if partition_name is not None:
            operands.append(bass2jax.partition_id_tensor())
        outs = bass2jax._bass_exec_p.bind(
            *operands,
            out_avals=tuple(out_avals),
            in_names=tuple(all_names),
            out_names=tuple(out_names),
            lowering_input_output_aliases=(),
            sim_require_finite=True,
            sim_require_nnan=True,
            nc=nc,
        )
        return tuple(outs)

    devices = jax.devices()[:N_CORES]
    mesh = Mesh(np.asarray(devices), ("core",))
    n_outs = len(out_names)
    sharded = jax.jit(
        shard_map(_body, mesh=mesh,
                  in_specs=(PartitionSpec("core"),) * (n_params + n_outs),
                  out_specs=(PartitionSpec("core"),) * n_outs,
                  check_rep=False),
        keep_unused=True,
    )
    sharding = NamedSharding(mesh, PartitionSpec("core"))
    runner = {
        "nc": nc, "sharded": sharded, "in_names": in_names,
        "out_names": out_names, "out_avals": out_avals,
        "zero_outs": zero_outs, "sharding": sharding, "jax": jax,
    }
    _CACHE[key] = runner
    return runner


def _stage(runner, in_maps):
    jax = runner["jax"]
    concat_in = [
        np.concatenate([np.asarray(in_maps[c][name]) for c in range(N_CORES)],
                       axis=0)
        for name in runner["in_names"]
    ]
    concat_zero = [
        np.zeros((N_CORES * z.shape[0], *z.shape[1:]), z.dtype)
        for z in runner["zero_outs"]
    ]
    return [jax.device_put(a, runner["sharding"]) for a in concat_in + concat_zero]


def _execute(runner, staged):
    jax = runner["jax"]
    outs = runner["sharded"](*staged)
    outs = jax.block_until_ready(outs)
    return outs


def kernel(hidden_states, position_ids, W_qkv, W_out):
    runner = _get_runner()
    in_maps = _in_maps(hidden_states, position_ids, W_qkv, W_out)
    staged = _stage(runner, in_maps)
    outs = _execute(runner, staged)
    yc = np.asarray(outs[0]).reshape(N_CORES, S, LOCAL)
    result = np.empty((B, S, D), dtype=np.float32)
    for c in range(N_CORES):
        dp, tp = c // MP, c % MP
        result[dp][:, tp * LOCAL:(tp + 1) * LOCAL] = yc[c]
    return result


def bench(inputs, iters=10, n_repeat=1):
    """Return per-call wall-clock seconds (list) for the staged executable."""
    import time
    runner = _get_runner(n_repeat)
    in_maps = _in_maps(**inputs)
    staged = _stage(runner, in_maps)
    _execute(runner, staged)  # warm-up / compile
    times = []
    for _ in range(iters):
        t0 = time.perf_counter()
        _execute(runner, staged)
        times.append(time.perf_counter() - t0)
    return times

